# revision 11
# baseline (speedup 1.0000x reference)
"""Causal self-attention (single head, d=1024) on 8 trn2 NeuronCores.

Problem: x [4, 2048, 1024] f32, W_qkv [1024, 3072] f32.
  qkv = x @ W_qkv; q,k,v = split(qkv); out = softmax(causal(q k^T / 32)) v.

Sharding: 8 shards = 4 batches x 2 interleaved query-shards. Core c handles
batch c//2 and q-blocks (c%2)::2 of the 16 128-row blocks. Interleaving
makes the causal work of the two shards of a batch identical: slot i
(i=0..7) processes q-block 2i+h against key blocks [0, 2i+2), with the
causal boundary encoded in a per-core additive mask over the last 2 key
blocks. Every core runs the same static program; only input data differs.

All matmuls run in bf16 (1 cyc/row on the PE; fp32 PSUM accumulation).
Inputs are cast to bf16 on the host, halving the input DMA. Measured
output error vs the fp32 reference is ~3e-3 relative, dominated by the
softmax-weight rounding, which bf16-vs-fp32r operand choice barely moves.

Per-core pipeline:
  A: Q^T[do,q] = Wq^T x_q^T     (x_q = shard's q rows, slot order)
  B: K^T[do,k] = Wk^T x^T
  C: V[k,do]   = (x^T)^T Wv
  D: per slot: S = Q^T.T K^T (psum, fp32) -> +mask -> exp(S/32) -> bf16 P
     + row-sums -> PE-transpose P -> out_psum += P^T.T V -> out * (1/sum).
Each phase's weight tile is prefetched during the previous phase via a
2-deep shared-tag pool; x is streamed in chunks.
"""

import sys

import numpy as np

for _p in ("/opt/trn_rl_repo", "/root/.axon_site/_ro/trn_rl_repo"):
    if _p not in sys.path:
        sys.path.append(_p)

import ml_dtypes
import concourse.bass as bass
import concourse.mybir as mybir
import concourse.tile as tile
from concourse.bass_utils import run_bass_kernel_spmd
from concourse.masks import make_identity

F32 = mybir.dt.float32
BF16 = mybir.dt.bfloat16

B, N, D = 4, 2048, 1024
DO = 1024
NB = N // 128      # 16 key blocks
SLOTS = NB // 2    # 8 q-blocks per core
QROWS = SLOTS * 128
SCALE = 1.0 / (DO ** 0.5)
NEG = -1.0e30
N_CORES = 8
DC = D // 128      # 8 contraction chunks

_CACHE = {}


def _split_multi_waits(nc, max_waits=1):
    """This walrus build allows one sync-wait per instruction; Tile attaches
    several. Hoist extras onto same-engine NoOps inserted just before."""
    ctr = 0
    for fn in nc.m.functions:
        for bb in fn.blocks:
            insts = bb.instructions
            if not any(
                i.sync_info and i.sync_info.on_wait and len(i.sync_info.on_wait) > max_waits
                for i in insts
            ):
                continue
            new_insts = []
            for inst in insts:
                si = inst.sync_info
                waits = list(si.on_wait) if (si and si.on_wait) else []
                if len(waits) > max_waits:
                    extra, keep = waits[:-max_waits], waits[-max_waits:]
                    for j in range(0, len(extra), max_waits):
                        nop = mybir.InstNoOp(name=f"I-ws-{ctr}", ins=[], outs=[])
                        ctr += 1
                        nop.engine = inst.engine
                        nop.sync_info = mybir.SyncInfo(
                            on_wait=extra[j:j + max_waits], on_update=[])
                        new_insts.append(nop)
                    si.on_wait = keep
                new_insts.append(inst)
            bb.instructions = new_insts


def _build():
    nc = bass.Bass()

    xq_t = nc.dram_tensor("xq", [D, QROWS], BF16, kind="ExternalInput")
    xt_t = nc.dram_tensor("xt", [D, N], BF16, kind="ExternalInput")
    w_t = nc.dram_tensor("w", [D, 3 * DO], BF16, kind="ExternalInput")
    mask_t = nc.dram_tensor("mask", [SLOTS, 128, 256], F32, kind="ExternalInput")
    out_t = nc.dram_tensor("out", [QROWS, DO], F32, kind="ExternalOutput")

    w_r = w_t[:].rearrange("(po pi) n -> pi po n", pi=128)    # [128, 8, 3072]
    xt_r = xt_t[:].rearrange("(po pi) k -> pi po k", pi=128)  # [128, 8, 2048]
    xq_r = xq_t[:].rearrange("(po pi) q -> pi po q", pi=128)  # [128, 8, 1024]

    with tile.TileContext(nc) as tc:
        with (
            tc.tile_pool(name="res", bufs=1) as res,
            tc.tile_pool(name="wpool", bufs=2) as wp,
            tc.tile_pool(name="xpool", bufs=3) as xp,
            tc.tile_pool(name="dwork", bufs=2) as dw,
            tc.tile_pool(name="psum", bufs=2, space="PSUM") as psA,
            tc.tile_pool(name="psum_av", bufs=4, space="PSUM") as psAV,
            tc.tile_pool(name="psum_tp", bufs=2, space="PSUM") as psTP,
        ):
            qt_sb = res.tile([128, DC, QROWS], BF16)   # Q^T  16KB/part
            kt_sb = res.tile([128, DC, N], BF16)       # K^T  32KB/part
            v_sb = res.tile([128, NB, DO], BF16)       # V    32KB/part
            mask_sb = res.tile([128, SLOTS, 256], F32)
            ident = res.tile([128, 128], BF16)
            make_identity(nc, ident[:])

            # ---- phase A: Q^T = Wq^T @ xq^T ------------------------------
            # per-dc DMA splits let the first matmuls start as soon as the
            # first 128-row stripes of Wq and xq land (cold-start pipelining)
            wq = wp.tile([128, DC, DO], BF16, tag="w", name="wq")
            for dc in range(DC):
                nc.sync.dma_start(wq[:, dc, :], w_r[:, dc, 0:DO])
            for qc in range(QROWS // 512):
                xq = xp.tile([128, DC, 512], BF16, tag="x", name=f"xq{qc}")
                for dc in range(DC):
                    nc.sync.dma_start(
                        xq[:, dc, :], xq_r[:, dc, qc * 512:(qc + 1) * 512])
                for ob in range(DC):
                    ps = psA.tile([128, 512], F32, tag="mm", name=f"psa{qc}_{ob}")
                    for dc in range(DC):
                        nc.tensor.matmul(
                            ps[:], wq[:, dc, ob * 128:(ob + 1) * 128],
                            xq[:, dc, :],
                            start=(dc == 0), stop=(dc == DC - 1))
                    nc.vector.tensor_copy(
                        qt_sb[:, ob, qc * 512:(qc + 1) * 512], ps[:])

            # ---- phase B: K^T = Wk^T @ x^T -------------------------------
            wk = wp.tile([128, DC, DO], BF16, tag="w", name="wk")
            nc.sync.dma_start(wk[:], w_r[:, :, DO:2 * DO])
            for kc in range(N // 512):
                xkc = xp.tile([128, DC, 512], BF16, tag="x", name=f"xkc{kc}")
                nc.sync.dma_start(xkc[:], xt_r[:, :, kc * 512:(kc + 1) * 512])
                for ob in range(DC):
                    ps = psA.tile([128, 512], F32, tag="mm", name=f"psb{kc}_{ob}")
                    for dc in range(DC):
                        nc.tensor.matmul(
                            ps[:], wk[:, dc, ob * 128:(ob + 1) * 128],
                            xkc[:, dc, :],
                            start=(dc == 0), stop=(dc == DC - 1))
                    nc.vector.tensor_copy(
                        kt_sb[:, ob, kc * 512:(kc + 1) * 512], ps[:])

            # ---- phase C: V = x @ Wv  (x tiles as stationary operand) ----
            wv = wp.tile([128, DC, DO], BF16, tag="w", name="wv")
            nc.sync.dma_start(wv[:], w_r[:, :, 2 * DO:3 * DO])
            nc.sync.dma_start(mask_sb[:], mask_t[:].rearrange("s p m -> p s m"))
            for kb in range(NB):
                xkb = xp.tile([128, DC, 512], BF16, tag="x", name=f"xkb{kb}")
                nc.sync.dma_start(
                    xkb[:, :, :128], xt_r[:, :, kb * 128:(kb + 1) * 128])
                for hf in range(2):
                    ps = psA.tile([128, 512], F32, tag="mm", name=f"psc{kb}_{hf}")
                    for dc in range(DC):
                        nc.tensor.matmul(
                            ps[:], xkb[:, dc, :128],
                            wv[:, dc, hf * 512:(hf + 1) * 512],
                            start=(dc == 0), stop=(dc == DC - 1))
                    nc.vector.tensor_copy(
                        v_sb[:, kb, hf * 512:(hf + 1) * 512], ps[:])

            # ---- phase D: attention per slot, big/small pairs so the two
            # in-flight slots always include one with enough PE work to hide
            # the other's scores->exp->transpose->AV serial chain ----------
            slot_order = []
            for j in range(SLOTS // 2):
                slot_order += [SLOTS - 1 - j, j]
            for i in slot_order:
                nk = 2 * i + 2                   # key blocks this slot
                ncols = nk * 128
                nch = (ncols + 511) // 512       # score chunks
                p_sb = dw.tile([128, N], BF16, tag="p", name=f"p{i}")
                sums = dw.tile([128, 4], F32, tag="sums", name=f"sums{i}")
                o_ps = [psAV.tile([128, 512], F32, tag="av", name=f"av{i}_{h}")
                        for h in range(2)]

                for kc in range(nch):
                    c0 = kc * 512
                    cw = min(512, ncols - c0)
                    ps = psA.tile([128, 512], F32, tag="mm", name=f"psd{i}_{kc}")
                    for dc in range(DC):
                        nc.tensor.matmul(
                            ps[:, :cw],
                            qt_sb[:, dc, i * 128:(i + 1) * 128],
                            kt_sb[:, dc, c0:c0 + cw],
                            start=(dc == 0), stop=(dc == DC - 1))
                    if kc == nch - 1:
                        # causal boundary: additive mask on last 2 blocks
                        nc.vector.tensor_add(
                            ps[:, cw - 256:cw], ps[:, cw - 256:cw],
                            mask_sb[:, i, :])
                    nc.scalar.activation(
                        p_sb[:, c0:c0 + cw], ps[:, :cw],
                        mybir.ActivationFunctionType.Exp,
                        scale=SCALE, accum_out=sums[:, kc:kc + 1])

                    for kb in range(c0 // 128, (c0 + cw) // 128):
                        tp = psTP.tile([128, 128], BF16, tag="tp",
                                       name=f"tp{i}_{kb}")
                        nc.tensor.transpose(
                            tp[:], p_sb[:, kb * 128:(kb + 1) * 128], ident[:])
                        pt = dw.tile([128, 128], BF16, tag="pt",
                                     name=f"pt{i}_{kb}")
                        nc.vector.tensor_copy(pt[:], tp[:])
                        for hf in range(2):
                            nc.tensor.matmul(
                                o_ps[hf][:], pt[:],
                                v_sb[:, kb, hf * 512:(hf + 1) * 512],
                                start=(kb == 0), stop=(kb == nk - 1))

                stot = dw.tile([128, 1], F32, tag="stot", name=f"st{i}")
                recip = dw.tile([128, 1], F32, tag="recip", name=f"rc{i}")
                nc.vector.reduce_sum(stot[:], sums[:, :nch],
                                     axis=mybir.AxisListType.X)
                nc.vector.reciprocal(recip[:], stot[:])
                o_sb = dw.tile([128, DO], F32, tag="osb", name=f"o{i}")
                for hf in range(2):
                    nc.vector.tensor_scalar_mul(
                        o_sb[:, hf * 512:(hf + 1) * 512], o_ps[hf][:], recip[:])
                nc.sync.dma_start(out_t[i * 128:(i + 1) * 128, :], o_sb[:])

    _split_multi_waits(nc)
    return nc


def _host_inputs(x, W_qkv):
    """Per-core input maps. Core c: batch c//2, q-blocks (c%2)::2."""
    bf = ml_dtypes.bfloat16
    in_maps = []
    perms = []
    w_bf = np.ascontiguousarray(W_qkv.astype(bf))
    for c in range(N_CORES):
        b, h = divmod(c, 2)
        blocks = list(range(h, NB, 2))
        qperm = np.concatenate(
            [np.arange(blk * 128, (blk + 1) * 128) for blk in blocks])
        perms.append((b, qperm))
        xb = x[b].astype(bf)                          # [N, D]
        xt = np.ascontiguousarray(xb.T)               # [D, N]
        xq = np.ascontiguousarray(xb[qperm].T)        # [D, QROWS]
        # additive causal mask for the last 2 key blocks of each slot:
        # slot i, q rows r (0..127) are global rows 256*i + 128*h + r; the
        # mask window covers global keys [256*i, 256*i + 256).
        mask = np.empty((SLOTS, 128, 256), np.float32)
        r = np.arange(128)[:, None]
        j = np.arange(256)[None, :]
        allow = j <= (128 * h + r)
        mask[:] = np.where(allow, 0.0, NEG)[None]
        in_maps.append({"xq": xq, "xt": xt, "w": w_bf, "mask": mask})
    return in_maps, perms


def kernel(x, W_qkv):
    x = np.asarray(x, dtype=np.float32)
    W_qkv = np.asarray(W_qkv, dtype=np.float32)
    if "nc" not in _CACHE:
        _CACHE["nc"] = _build()
    nc = _CACHE["nc"]
    in_maps, perms = _host_inputs(x, W_qkv)
    res = run_bass_kernel_spmd(nc, in_maps, core_ids=list(range(N_CORES)))
    out = np.empty((B, N, DO), np.float32)
    for c, (b, qperm) in enumerate(perms):
        out[b, qperm] = res.results[c]["out"]
    return out
